# revision 8
# baseline (speedup 1.0000x reference)
"""Trainium2 Bass kernel for nn_LittleBitParallelLinear.

Computes y = ((x * h_in) @ sign(V)) * s @ sign(U).T * h_out with
sign(z) = +1 for z >= 0, -1 otherwise.

Strategy: token-parallel across 8 NeuronCores. Core i handles tokens
[i*1024, (i+1)*1024); weights are replicated. Inside each core everything
is computed transposed (tokens on the matmul free dim) so that h_in, s and
h_out all become per-partition scales:

    aT  = (xT * h_in)            [IN, TOK]   bf16, SBUF-resident
    tT  = (sign(V).T @ aT) * s   [RANK, TOK] bf16, SBUF-resident
    yT  = (sign(U) @ tT) * h_out [OUT, TOK]  fp32, streamed to DRAM

Matmuls run in bf16 (sign weights are exactly representable; activations
round to ~0.4% rel err). The host pre-transposes x and u and casts the
big tensors to bf16 so DMA traffic is halved; the sign() itself is
computed on-device.
"""

import numpy as np
import ml_dtypes

P = 128
TOKENS, IN, OUT, RANK = 8192, 4096, 4096, 2048
N_CORES = 8
TOK = TOKENS // N_CORES  # tokens per core
KI = IN // P             # 32 contraction subtiles for mm1
KR = RANK // P           # 16 contraction subtiles for mm2
MR = RANK // P           # 16 rank row-blocks (mm1 output)
MO = OUT // P            # 32 out row-blocks (mm2 output)
FREE = 512               # PSUM bank free-dim (fp32)
NT = TOK // FREE         # 2 free chunks of the token dim

_cache = {}


def _build(reps=1):
    import concourse.bacc as bacc
    import concourse.mybir as mybir
    import concourse.tile as tile

    f32 = mybir.dt.float32
    bf16 = mybir.dt.bfloat16
    Sign = mybir.ActivationFunctionType.Sign
    Copy = mybir.ActivationFunctionType.Copy

    nc = bacc.Bacc("TRN2", target_bir_lowering=False, debug=False)

    xT = nc.dram_tensor("xT", [IN, TOK], bf16, kind="ExternalInput").ap()
    # weights arrive pre-tiled: block m is contiguous [P, K_sub, P]
    v_ = nc.dram_tensor("v", [MR, P, KI, P], bf16, kind="ExternalInput").ap()
    uT = nc.dram_tensor("uT", [MO, P, KR, P], bf16, kind="ExternalInput").ap()
    s_ = nc.dram_tensor("s", [P, KR], f32, kind="ExternalInput").ap()
    hi = nc.dram_tensor("h_in", [P, KI], f32, kind="ExternalInput").ap()
    ho = nc.dram_tensor("h_out", [P, MO], f32, kind="ExternalInput").ap()
    yT = nc.dram_tensor("yT", [OUT, TOK], f32, kind="ExternalOutput").ap()

    with tile.TileContext(nc) as tc:
      for rep in range(reps):
        with (
            tc.tile_pool(name=f"const{rep}", bufs=1) as const,
            tc.tile_pool(name=f"aT{rep}", bufs=1) as apool,
            tc.tile_pool(name=f"tT{rep}", bufs=1) as tpool,
            tc.tile_pool(name=f"xin{rep}", bufs=3) as xpool,
            tc.tile_pool(name=f"vin{rep}", bufs=3) as vpool,
            tc.tile_pool(name=f"bv{rep}", bufs=4) as bvpool,
            tc.tile_pool(name=f"uin{rep}", bufs=2) as upool,
            tc.tile_pool(name=f"bu{rep}", bufs=2) as bupool,
            tc.tile_pool(name=f"yout{rep}", bufs=3) as ypool,
            tc.tile_pool(name=f"psum{rep}", bufs=6, space="PSUM") as psum,
        ):

            # aT = xT * h_in, bf16, fully SBUF-resident [P, KI, TOK]
            # Interleave the x-tile loads with the v-weight loads in issue
            # order so the first weight blocks aren't queued behind all of x.
            aT = apool.tile([P, KI, TOK], bf16)
            x3 = xT.rearrange("(ko p) t -> p ko t", p=P)

            bv_tiles = {}

            def load_bv(m):
                vt = vpool.tile([P, KI, P], bf16, name=f"vt{rep}_{m}", tag="vt")
                nc.sync.dma_start(vt, v_[m])
                bv = bvpool.tile([P, KI, P], bf16, name=f"bv{rep}_{m}", tag="bv")
                for c in range(0, KI, 8):
                    nc.scalar.activation(bv[:, c : c + 8], vt[:, c : c + 8], Sign)
                bv_tiles[m] = bv

            load_bv(0)
            load_bv(1)
            # consts: pre-tiled on host, contiguous small DMAs
            hi_sb = const.tile([P, KI], f32)
            nc.sync.dma_start(hi_sb, hi)
            s_sb = const.tile([P, KR], f32)
            nc.sync.dma_start(s_sb, s_)
            ho_sb = const.tile([P, MO], f32)
            nc.sync.dma_start(ho_sb, ho)
            for k in range(KI):
                xt = xpool.tile([P, TOK], bf16)
                nc.sync.dma_start(xt, x3[:, k])
                nc.vector.tensor_scalar_mul(aT[:, k], xt, hi_sb[:, k : k + 1])

            # tT = (sign(V).T @ aT) * s, bf16, SBUF-resident [P, KR, TOK]
            tT = tpool.tile([P, KR, TOK], bf16)
            for m in range(MR):
                if 2 + m <= MR - 1:
                    load_bv(2 + m)
                bv = bv_tiles.pop(m)
                pss = [
                    psum.tile([P, FREE], f32, name=f"ps1_{rep}_{m}_{n}", tag="ps")
                    for n in range(NT)
                ]
                for k in range(KI):
                    for n in range(NT):
                        nc.tensor.matmul(
                            pss[n],
                            lhsT=bv[:, k],
                            rhs=aT[:, k, n * FREE : (n + 1) * FREE],
                            start=(k == 0),
                            stop=(k == KI - 1),
                        )
                for n in range(NT):
                    nc.scalar.activation(
                        tT[:, m, n * FREE : (n + 1) * FREE],
                        pss[n],
                        Copy,
                        scale=s_sb[:, m : m + 1],
                    )

            # yT = (sign(U) @ tT) * h_out, fp32, streamed out
            y3 = yT.rearrange("(mo p) t -> p mo t", p=P)
            for m in range(MO):
                ut = upool.tile([P, KR, P], bf16)
                nc.sync.dma_start(ut, uT[m])
                bu = bupool.tile([P, KR, P], bf16)
                for c in range(0, KR, 8):
                    nc.scalar.activation(bu[:, c : c + 8], ut[:, c : c + 8], Sign)
                pss = [
                    psum.tile([P, FREE], f32, name=f"ps2_{rep}_{m}_{n}", tag="ps")
                    for n in range(NT)
                ]
                for k in range(KR):
                    for n in range(NT):
                        nc.tensor.matmul(
                            pss[n],
                            lhsT=bu[:, k],
                            rhs=tT[:, k, n * FREE : (n + 1) * FREE],
                            start=(k == 0),
                            stop=(k == KR - 1),
                        )
                yst = ypool.tile([P, TOK], f32)
                for n in range(NT):
                    nc.scalar.activation(
                        yst[:, n * FREE : (n + 1) * FREE],
                        pss[n],
                        Copy,
                        scale=ho_sb[:, m : m + 1],
                    )
                nc.sync.dma_start(y3[:, m], yst)

    nc.compile()
    return nc


def _run(inputs, trace=False):
    from concourse.bass_utils import run_bass_kernel_spmd

    if "nc" not in _cache:
        _cache["nc"] = _build()
    nc = _cache["nc"]

    x = inputs["x"]
    u = inputs["u"]
    v = inputs["v"]
    def ptile(vec, o):
        return np.ascontiguousarray(
            np.asarray(vec, dtype=np.float32).reshape(o, P).T
        )

    s = ptile(inputs["s"], KR)
    h_in = ptile(inputs["h_in"], KI)
    h_out = ptile(inputs["h_out"], MO)

    bf = ml_dtypes.bfloat16
    # pre-tile weights so each 128-wide block is a contiguous DMA:
    # v_t[m, p, k, r] = v[k*128+p, m*128+r]; u_t[m, p, k, o] = u[m*128+o, k*128+p]
    v_bf = np.ascontiguousarray(
        np.asarray(v).reshape(KI, P, MR, P).transpose(2, 1, 0, 3)
    ).astype(bf)
    uT_bf = np.ascontiguousarray(
        np.asarray(u).T.reshape(KR, P, MO, P).transpose(2, 1, 0, 3)
    ).astype(bf)

    in_maps = []
    for i in range(N_CORES):
        xT_i = np.ascontiguousarray(x[i * TOK : (i + 1) * TOK, :].T).astype(bf)
        in_maps.append(
            {
                "xT": xT_i,
                "v": v_bf,
                "uT": uT_bf,
                "s": s,
                "h_in": h_in,
                "h_out": h_out,
            }
        )

    res = run_bass_kernel_spmd(
        nc, in_maps, core_ids=list(range(N_CORES)), trace=trace
    )

    y = np.empty((TOKENS, OUT), dtype=np.float32)
    for i in range(N_CORES):
        y[i * TOK : (i + 1) * TOK, :] = res.results[i]["yT"].T
    return y, res


def kernel(**inputs):
    y, _ = _run(inputs, trace=False)
    return y
